# revision 22
# baseline (speedup 1.0000x reference)
"""MoE layer (nn_MoELayer_4681514353281) Trainium2 Bass kernel.

Reference semantics: for slot i in range(4), expert i's FFN (W1 + A1@B1 LoRA,
gelu-tanh, W2 + A2@B2 LoRA) runs densely over ALL tokens; per-token combine
weight = renormalized top-4 softmax gate weight where top_idx == i (else 0).
Only experts 0-3 are ever used.

Sharding: 8 cores = 4 experts x 2 halves of F (expert-parallel + intra-expert
F-split). Each core computes w_e[t] * gelu(x @ W1c) @ W2c for all 8192 tokens
on its F-half, where W1c = W1 + A1@B1 and W2c = W2 + A2@B2 are the LoRA-folded
dense weights (exact algebraic identity, folded on host). Host sums the 8
partial [8192, 1024] outputs.

The gate's top-4 selection needs ~1e-6 logit precision to reproduce the fp32
reference's picks (near-ties flip otherwise), which exceeds the PE's FP22
multiply path; the 8192x16 softmax/top-4 (0.13% of FLOPs) is computed on the
host and shipped as a [128, 64] weight table. All FFN compute runs on device
in float32r at full PE rate.
"""

import os
import sys

sys.path.insert(0, "/opt/trn_rl_repo")

import numpy as np

# Problem dims (hardcoded per spec)
B, S, D, F, E, R = 2, 4096, 1024, 4096, 16, 16
TOPK = 4
N_TOK = B * S          # 8192
F2 = F // 2            # 2048 per-core F half
TOK_BLK = 512
DC = D // 128          # 8
FC = F2 // 128         # 16
SUBS = TOK_BLK // 128  # 4

_programs = {}
LAST_RESULTS = None


def _build_program(n_blk):
    import concourse.tile as tile
    from concourse import bacc, mybir

    F32R = mybir.dt.float32r
    F32 = mybir.dt.float32
    AF = mybir.ActivationFunctionType

    nc = bacc.Bacc("TRN2", target_bir_lowering=False, debug=False, num_devices=8)

    xTd = nc.dram_tensor("xT", [D, N_TOK], F32R, kind="ExternalInput")
    w1d = nc.dram_tensor("w1", [D, F2], F32R, kind="ExternalInput")
    w2d = nc.dram_tensor("w2", [F2, D], F32R, kind="ExternalInput")
    wcd = nc.dram_tensor("wc", [128, N_TOK // 128], F32, kind="ExternalInput")
    outd = nc.dram_tensor("out", [N_TOK, D], F32, kind="ExternalOutput")

    with tile.TileContext(nc) as tc:
        with (
            tc.tile_pool(name="singles", bufs=1) as singles,
            tc.tile_pool(name="xp", bufs=2) as xp,
            tc.tile_pool(name="hap", bufs=FC + 2) as hap,
            tc.tile_pool(name="outp", bufs=3) as outp,
            tc.tile_pool(name="psH", bufs=3, space="PSUM") as psH,
            tc.tile_pool(name="psEO", bufs=5, space="PSUM") as psEO,
        ):
            # ---- resident weights ----
            w1 = singles.tile([128, FC, DC, 128], F32R)   # [p, fc, dc, q]
            w2 = singles.tile([128, FC, D], F32R)         # [p, fc, d]
            w_all = singles.tile([128, N_TOK // 128], F32)

            xT_r = xTd.rearrange("(dc p) t -> p dc t", p=128)
            w1_r = w1d.rearrange("(dc p) (fc q) -> p fc dc q", p=128, q=128)
            w2_r = w2d.rearrange("(fc p) d -> p fc d", p=128)

            def load_block(b, split=False):
                t = xp.tile([128, DC, TOK_BLK], F32R, tag="xb")
                sl = slice(b * TOK_BLK, (b + 1) * TOK_BLK)
                if split:
                    for dc in range(DC):
                        nc.scalar.dma_start(t[:, dc, :], xT_r[:, dc, sl])
                else:
                    nc.scalar.dma_start(t[:], xT_r[:, :, sl])
                return t

            xb = load_block(0, split=True)

            # fc0's w1 chunks arrive first (per-dc) so the PE starts early
            for dc in range(DC):
                nc.sync.dma_start(w1[:, 0, dc, :], w1_r[:, 0, dc, :])
            nc.sync.dma_start(w_all[:], wcd[:, :])
            for fc in range(FC):
                if fc > 0:
                    nc.sync.dma_start(w1[:, fc, :, :], w1_r[:, fc, :, :])
                nc.sync.dma_start(w2[:, fc, :], w2_r[:, fc, :])

            for blk in range(n_blk):
                # up projection: h[fc][:, t] = gelu(x @ W1c)[f, t]
                h_all = []
                for fc in range(FC):
                    ps_h = psH.tile([128, TOK_BLK], F32)
                    for dc in range(DC):
                        nc.tensor.matmul(
                            ps_h[:], w1[:, fc, dc, :], xb[:, dc, :],
                            start=(dc == 0), stop=(dc == DC - 1),
                        )
                    h = hap.tile([128, TOK_BLK], F32R, tag="h")
                    nc.scalar.activation(h[:], ps_h[:], AF.Gelu_apprx_tanh)
                    h_all.append(h)

                # prefetch next block's x while the down passes run
                if blk + 1 < n_blk:
                    xb_next = load_block(blk + 1)
                else:
                    xb_next = None

                # down projection in two d-half passes, 128-token columns
                for dh in range(2):
                    for sub in range(SUBS):
                        eo = psEO.tile([128, 512], F32, tag="eo")
                        for fc in range(FC):
                            nc.tensor.matmul(
                                eo[:],
                                h_all[fc][:, sub * 128:(sub + 1) * 128],
                                w2[:, fc, dh * 512:(dh + 1) * 512],
                                start=(fc == 0), stop=(fc == FC - 1),
                            )
                        ob = outp.tile([128, 512], F32, tag="ob")
                        col = SUBS * blk + sub
                        nc.vector.tensor_scalar_mul(
                            ob[:], eo[:], scalar1=w_all[:, col:col + 1]
                        )
                        t0 = blk * TOK_BLK + sub * 128
                        nc.scalar.dma_start(
                            outd[t0:t0 + 128, dh * 512:(dh + 1) * 512], ob[:]
                        )

                xb = xb_next

    nc.compile()
    return nc


def _get_program(n_blk):
    if n_blk not in _programs:
        _programs[n_blk] = _build_program(n_blk)
    return _programs[n_blk]


def _gate_weights(x2d, Wg):
    """Reference-faithful gate (same ops as the reference, jax on CPU so the
    fp32 softmax/top-4 selection matches bit-for-bit). Returns [N_TOK, 4]
    combine weights for experts 0-3."""
    try:
        import jax
        import jax.numpy as jnp
        cpu = jax.devices("cpu")[0]
        with jax.default_device(cpu):
            xf = jnp.asarray(x2d, jnp.float32)
            wg = jnp.asarray(Wg, jnp.float32)
            weights = jax.nn.softmax(xf @ wg, axis=-1)
            top_w, top_idx = jax.lax.top_k(weights, TOPK)
            top_w = top_w / jnp.sum(top_w, axis=-1, keepdims=True)
            cols = [jnp.sum(top_w * (top_idx == i), axis=-1) for i in range(TOPK)]
            return np.asarray(jnp.stack(cols, axis=-1), np.float32)
    except Exception:
        # numpy fallback (identical math, BLAS rounding may differ ~1e-7)
        logits = x2d.astype(np.float32) @ Wg.astype(np.float32)
        m = logits.max(axis=-1, keepdims=True)
        e = np.exp((logits - m).astype(np.float32), dtype=np.float32)
        p = (e / e.sum(axis=-1, keepdims=True).astype(np.float32)).astype(np.float32)
        idx = np.argsort(-p, axis=-1, kind="stable")[:, :TOPK]
        topw = np.take_along_axis(p, idx, axis=-1)
        topw = (topw / topw.sum(axis=-1, keepdims=True)).astype(np.float32)
        w = np.zeros((x2d.shape[0], TOPK), np.float32)
        for i in range(TOPK):
            w[:, i] = (topw * (idx == i)).sum(axis=-1)
        return w


def kernel(x, Wg, W1, A1, B1, W2, A2, B2):
    global LAST_RESULTS
    from concourse.bass_utils import run_bass_kernel_spmd

    n_blk = int(os.environ.get("KERNEL_NBLK", N_TOK // TOK_BLK))
    nc = _get_program(n_blk)

    x = np.asarray(x, dtype=np.float32)
    x2d = x.reshape(N_TOK, D)
    xT = np.ascontiguousarray(x2d.T)
    w4 = _gate_weights(x2d, np.asarray(Wg, dtype=np.float32))

    in_maps = []
    for core in range(8):
        e = core % 4
        half = core // 4
        f0, f1 = half * F2, (half + 1) * F2
        # fold the rank-16 LoRA into the dense weights (exact identity)
        w1c = (np.asarray(W1[e], np.float64)
               + np.asarray(A1[e], np.float64) @ np.asarray(B1[e], np.float64))
        w2c = (np.asarray(W2[e], np.float64)
               + np.asarray(A2[e], np.float64) @ np.asarray(B2[e], np.float64))
        # [128, N_TOK//128]: column c holds tokens [c*128, (c+1)*128)
        wc = np.ascontiguousarray(w4[:, e].reshape(N_TOK // 128, 128).T)
        in_maps.append({
            "xT": xT,
            "w1": np.ascontiguousarray(w1c[:, f0:f1], dtype=np.float32),
            "w2": np.ascontiguousarray(w2c[f0:f1, :], dtype=np.float32),
            "wc": wc,
        })

    trace = bool(os.environ.get("KERNEL_TRACE"))
    res = None
    last_exc = None
    for _attempt in range(3):
        try:
            res = run_bass_kernel_spmd(
                nc, in_maps, core_ids=list(range(8)), trace=trace
            )
            break
        except Exception as exc:  # transient NRT/profiling faults — retry
            last_exc = exc
            trace = False
    if res is None:
        raise last_exc
    LAST_RESULTS = res

    out = res.results[0]["out"].astype(np.float64)
    for core in range(1, 8):
        out += res.results[core]["out"]
    return out.astype(np.float32).reshape(B, S, D)


# revision 23
# speedup vs baseline: 1.0039x; 1.0039x over previous
"""MoE layer (nn_MoELayer_4681514353281) Trainium2 Bass kernel.

Reference semantics: for slot i in range(4), expert i's FFN (W1 + A1@B1 LoRA,
gelu-tanh, W2 + A2@B2 LoRA) runs densely over ALL tokens; per-token combine
weight = renormalized top-4 softmax gate weight where top_idx == i (else 0).
Only experts 0-3 are ever used.

Sharding: 8 cores = 4 experts x 2 halves of F (expert-parallel + intra-expert
F-split). Each core computes w_e[t] * gelu(x @ W1c) @ W2c for all 8192 tokens
on its F-half, where W1c = W1 + A1@B1 and W2c = W2 + A2@B2 are the LoRA-folded
dense weights (exact algebraic identity, folded on host). Host sums the 8
partial [8192, 1024] outputs.

The gate's top-4 selection needs ~1e-6 logit precision to reproduce the fp32
reference's picks (near-ties flip otherwise), which exceeds the PE's FP22
multiply path; the 8192x16 softmax/top-4 (0.13% of FLOPs) is computed on the
host and shipped as a [128, 64] weight table. All FFN compute runs on device
in float32r at full PE rate.
"""

import os
import sys

sys.path.insert(0, "/opt/trn_rl_repo")

import numpy as np

# Problem dims (hardcoded per spec)
B, S, D, F, E, R = 2, 4096, 1024, 4096, 16, 16
TOPK = 4
N_TOK = B * S          # 8192
F2 = F // 2            # 2048 per-core F half
TOK_BLK = 512
DC = D // 128          # 8
FC = F2 // 128         # 16
SUBS = TOK_BLK // 128  # 4

_programs = {}
LAST_RESULTS = None


def _build_program(n_blk):
    import concourse.tile as tile
    from concourse import bacc, mybir

    F32R = mybir.dt.float32r
    F32 = mybir.dt.float32
    AF = mybir.ActivationFunctionType

    nc = bacc.Bacc("TRN2", target_bir_lowering=False, debug=False, num_devices=8)

    xTd = nc.dram_tensor("xT", [D, N_TOK], F32R, kind="ExternalInput")
    w1d = nc.dram_tensor("w1", [D, F2], F32R, kind="ExternalInput")
    w2d = nc.dram_tensor("w2", [F2, D], F32R, kind="ExternalInput")
    wcd = nc.dram_tensor("wc", [128, N_TOK // 128], F32, kind="ExternalInput")
    outd = nc.dram_tensor("out", [N_TOK, D], F32, kind="ExternalOutput")

    with tile.TileContext(nc) as tc:
        with (
            tc.tile_pool(name="singles", bufs=1) as singles,
            tc.tile_pool(name="xp", bufs=2) as xp,
            tc.tile_pool(name="hap", bufs=FC + 2) as hap,
            tc.tile_pool(name="outp", bufs=3) as outp,
            tc.tile_pool(name="psH", bufs=3, space="PSUM") as psH,
            tc.tile_pool(name="psEO", bufs=5, space="PSUM") as psEO,
        ):
            # ---- resident weights ----
            w1 = singles.tile([128, FC, DC, 128], F32R)   # [p, fc, dc, q]
            w2 = singles.tile([128, FC, D], F32R)         # [p, fc, d]
            w_all = singles.tile([128, N_TOK // 128], F32)

            xT_r = xTd.rearrange("(dc p) t -> p dc t", p=128)
            w1_r = w1d.rearrange("(dc p) (fc q) -> p fc dc q", p=128, q=128)
            w2_r = w2d.rearrange("(fc p) d -> p fc d", p=128)

            def load_block(b):
                t = xp.tile([128, DC, TOK_BLK], F32R, tag="xb")
                nc.scalar.dma_start(
                    t[:], xT_r[:, :, b * TOK_BLK:(b + 1) * TOK_BLK]
                )
                return t

            xb = load_block(0)

            nc.sync.dma_start(w_all[:], wcd[:, :])
            for fc in range(FC):
                nc.sync.dma_start(w1[:, fc, :, :], w1_r[:, fc, :, :])
                nc.sync.dma_start(w2[:, fc, :], w2_r[:, fc, :])

            for blk in range(n_blk):
                # up projection: h[fc][:, t] = gelu(x @ W1c)[f, t]
                h_all = []
                for fc in range(FC):
                    ps_h = psH.tile([128, TOK_BLK], F32)
                    for dc in range(DC):
                        nc.tensor.matmul(
                            ps_h[:], w1[:, fc, dc, :], xb[:, dc, :],
                            start=(dc == 0), stop=(dc == DC - 1),
                        )
                    h = hap.tile([128, TOK_BLK], F32R, tag="h")
                    nc.scalar.activation(h[:], ps_h[:], AF.Gelu_apprx_tanh)
                    h_all.append(h)

                # prefetch next block's x while the down passes run
                if blk + 1 < n_blk:
                    xb_next = load_block(blk + 1)
                else:
                    xb_next = None

                # down projection in two d-half passes, 128-token columns
                for dh in range(2):
                    for sub in range(SUBS):
                        eo = psEO.tile([128, 512], F32, tag="eo")
                        for fc in range(FC):
                            nc.tensor.matmul(
                                eo[:],
                                h_all[fc][:, sub * 128:(sub + 1) * 128],
                                w2[:, fc, dh * 512:(dh + 1) * 512],
                                start=(fc == 0), stop=(fc == FC - 1),
                            )
                        ob = outp.tile([128, 512], F32, tag="ob")
                        col = SUBS * blk + sub
                        nc.vector.tensor_scalar_mul(
                            ob[:], eo[:], scalar1=w_all[:, col:col + 1]
                        )
                        t0 = blk * TOK_BLK + sub * 128
                        nc.scalar.dma_start(
                            outd[t0:t0 + 128, dh * 512:(dh + 1) * 512], ob[:]
                        )

                xb = xb_next

    nc.compile()
    return nc


def _get_program(n_blk):
    if n_blk not in _programs:
        _programs[n_blk] = _build_program(n_blk)
    return _programs[n_blk]


def _gate_weights(x2d, Wg):
    """Reference-faithful gate (same ops as the reference, jax on CPU so the
    fp32 softmax/top-4 selection matches bit-for-bit). Returns [N_TOK, 4]
    combine weights for experts 0-3."""
    try:
        import jax
        import jax.numpy as jnp
        cpu = jax.devices("cpu")[0]
        with jax.default_device(cpu):
            xf = jnp.asarray(x2d, jnp.float32)
            wg = jnp.asarray(Wg, jnp.float32)
            weights = jax.nn.softmax(xf @ wg, axis=-1)
            top_w, top_idx = jax.lax.top_k(weights, TOPK)
            top_w = top_w / jnp.sum(top_w, axis=-1, keepdims=True)
            cols = [jnp.sum(top_w * (top_idx == i), axis=-1) for i in range(TOPK)]
            return np.asarray(jnp.stack(cols, axis=-1), np.float32)
    except Exception:
        # numpy fallback (identical math, BLAS rounding may differ ~1e-7)
        logits = x2d.astype(np.float32) @ Wg.astype(np.float32)
        m = logits.max(axis=-1, keepdims=True)
        e = np.exp((logits - m).astype(np.float32), dtype=np.float32)
        p = (e / e.sum(axis=-1, keepdims=True).astype(np.float32)).astype(np.float32)
        idx = np.argsort(-p, axis=-1, kind="stable")[:, :TOPK]
        topw = np.take_along_axis(p, idx, axis=-1)
        topw = (topw / topw.sum(axis=-1, keepdims=True)).astype(np.float32)
        w = np.zeros((x2d.shape[0], TOPK), np.float32)
        for i in range(TOPK):
            w[:, i] = (topw * (idx == i)).sum(axis=-1)
        return w


def kernel(x, Wg, W1, A1, B1, W2, A2, B2):
    global LAST_RESULTS
    from concourse.bass_utils import run_bass_kernel_spmd

    n_blk = int(os.environ.get("KERNEL_NBLK", N_TOK // TOK_BLK))
    nc = _get_program(n_blk)

    x = np.asarray(x, dtype=np.float32)
    x2d = x.reshape(N_TOK, D)
    xT = np.ascontiguousarray(x2d.T)
    w4 = _gate_weights(x2d, np.asarray(Wg, dtype=np.float32))

    in_maps = []
    for core in range(8):
        e = core % 4
        half = core // 4
        f0, f1 = half * F2, (half + 1) * F2
        # fold the rank-16 LoRA into the dense weights (exact identity)
        w1c = (np.asarray(W1[e], np.float64)
               + np.asarray(A1[e], np.float64) @ np.asarray(B1[e], np.float64))
        w2c = (np.asarray(W2[e], np.float64)
               + np.asarray(A2[e], np.float64) @ np.asarray(B2[e], np.float64))
        # [128, N_TOK//128]: column c holds tokens [c*128, (c+1)*128)
        wc = np.ascontiguousarray(w4[:, e].reshape(N_TOK // 128, 128).T)
        in_maps.append({
            "xT": xT,
            "w1": np.ascontiguousarray(w1c[:, f0:f1], dtype=np.float32),
            "w2": np.ascontiguousarray(w2c[f0:f1, :], dtype=np.float32),
            "wc": wc,
        })

    trace = bool(os.environ.get("KERNEL_TRACE"))
    res = None
    last_exc = None
    for _attempt in range(3):
        try:
            res = run_bass_kernel_spmd(
                nc, in_maps, core_ids=list(range(8)), trace=trace
            )
            break
        except Exception as exc:  # transient NRT/profiling faults — retry
            last_exc = exc
            trace = False
    if res is None:
        raise last_exc
    LAST_RESULTS = res

    out = res.results[0]["out"].astype(np.float64)
    for core in range(1, 8):
        out += res.results[core]["out"]
    return out.astype(np.float32).reshape(B, S, D)
